# revision 13
# baseline (speedup 1.0000x reference)
"""Trainium2 Bass kernel for pointnet2-style ball_query (radius=3.4, nsample=5).

Input : x [8, 4096, 3] f32.
Output: [8, 4096, 5] int32 - for each query q the first 5 point indices k (in
scan order) with ||x_q - x_k||^2 < r^2; missing slots hold the first hit.

Strategy (data-parallel, one batch per NeuronCore):
  - PE matmul with augmented K=4 operands computes m[q,k] = sq_k - 2<x_q,x_k>
    for a 128-query tile x W-column window straight into PSUM.
  - ACT evacuates PSUM with Sign(-m + (r^2 - sq_q)) using a per-partition bias,
    giving the hit indicator in {-1, 0, +1}; a second ACT (Relu + accum_out)
    yields the per-row hit count in the window.
  - One DVE max_index instruction matching eight 1.0s returns the first 8 hit
    positions per row in scan order - the whole selection in a single op.
  - Tiny epilogue: slot j (j>0) falls back to the first hit when count <= j.
Rows are only correct if they have >= 5 hits inside the window, so the kernel
also emits per-row hit counts; the host re-runs a full-width (W=4096) variant
of the same kernel for any batch where some row has < 5 window hits.  (Every
row always has >= 1 hit at full width: the point itself.)
"""

import numpy as np

import concourse.bass as bass
import concourse.bacc as bacc
import concourse.mybir as mybir
from concourse.tile import TileContext
from concourse.bass_utils import run_bass_kernel_spmd

N = 4096          # points per batch
B = 8             # batches == cores
P = 128           # partitions (query tile height)
NT = N // P       # 32 query tiles
NS = 5            # nsample
W_FAST = 256      # scan window of the fast kernel (min hits on data: 13)
R2 = float(np.float32(3.4 * 3.4))

F32 = mybir.dt.float32
I32 = mybir.dt.int32
U32 = mybir.dt.uint32
AF = mybir.ActivationFunctionType
OP = mybir.AluOpType


def _build(w: int) -> bass.Bass:
    """Build the single-core program scanning the first `w` columns."""
    assert w % P == 0
    kchunk = min(w, 512)             # PSUM tile width (<= 1 bank x 512 f32)
    nk = w // kchunk

    nc = bacc.Bacc("TRN2", target_bir_lowering=False, debug=False)
    x_in = nc.dram_tensor("x", [N, 3], F32, kind="ExternalInput").ap()
    out_d = nc.dram_tensor("out", [N, NS], I32, kind="ExternalOutput").ap()
    cnt_d = nc.dram_tensor("cnt", [P, NT], F32, kind="ExternalOutput").ap()

    with TileContext(nc) as tc:
        with (
            tc.tile_pool(name="const", bufs=1) as cp,
            tc.tile_pool(name="psum", bufs=8, space="PSUM") as pp,
            tc.tile_pool(name="work", bufs=4) as wp,
        ):
            # ---- setup -----------------------------------------------------
            # x in query-tile-major layout: xq[p, t, d] = x[t*128 + p, d]
            xq = cp.tile([P, NT, 3], F32)
            nc.sync.dma_start(out=xq, in_=x_in.rearrange("(t p) d -> p t d", p=P))
            # x transposed: xT[d, q] = x[q, d]
            xT = cp.tile([3, N], F32)
            nc.sync.dma_start(out=xT, in_=x_in.rearrange("q d -> d q"))

            # first w points, row layout on one partition (for sq_k row)
            xrow = cp.tile([1, w, 3], F32)
            nc.sync.dma_start(out=xrow, in_=x_in[0:w, :].rearrange("k d -> (k d)"))

            # sq[p, t] = |x_q|^2 for q = t*128+p
            xsq = cp.tile([P, NT, 3], F32)
            nc.scalar.activation(xsq, xq, AF.Square)
            sqt = cp.tile([P, NT], F32)
            nc.vector.tensor_add(sqt, xsq[:, :, 0], xsq[:, :, 1])
            nc.vector.tensor_add(sqt, sqt, xsq[:, :, 2])
            # bias2[p, t] = (r^2 - sq_q) / 2  (per-partition ACT bias)
            biasT = cp.tile([P, NT], F32)
            nc.vector.tensor_scalar(biasT, sqt, -0.5, 0.5 * R2, op0=OP.mult, op1=OP.add)

            # ones row (K=1 lhsT for the -sq_k/2 accumulation matmul)
            onesrow = cp.tile([1, N], F32)
            nc.vector.memset(onesrow, 1.0)
            # msqrow[0, k] = -sq_k / 2 for k < w
            xrsq = cp.tile([1, w, 3], F32)
            nc.scalar.activation(xrsq, xrow, AF.Square)
            msqrow = cp.tile([1, w], F32)
            nc.vector.tensor_add(msqrow, xrsq[:, :, 0], xrsq[:, :, 1])
            nc.vector.tensor_add(msqrow, msqrow, xrsq[:, :, 2])
            nc.vector.tensor_scalar_mul(msqrow, msqrow, -0.5)

            ones8 = cp.tile([P, 8], F32)
            nc.vector.memset(ones8, 1.0)

            idx = cp.tile([P, NT, 8], U32)   # first-8 hit positions per row
            cnt = cp.tile([P, NT], F32)      # window hit count per row

            # ---- main loop: one 128-query tile at a time -------------------
            for t in range(NT):
                ind = wp.tile([P, w], F32, tag="ind")
                for c in range(nk):
                    ps = pp.tile([P, kchunk], F32, tag="ps")
                    ksl = slice(c * kchunk, (c + 1) * kchunk)
                    # ps = <x_q, x_k>
                    nc.tensor.matmul(
                        ps,
                        xT[:, t * P : (t + 1) * P],
                        xT[:, ksl],
                        start=True,
                        stop=False,
                    )
                    # ps += -sq_k/2
                    nc.tensor.matmul(
                        ps,
                        onesrow[:, t * P : (t + 1) * P],
                        msqrow[:, ksl],
                        start=False,
                        stop=True,
                    )
                    # ind = sign(<x_q,x_k> - sq_k/2 + (r^2 - sq_q)/2)
                    #     = sign((r^2 - d2)/2) : +1 exactly at hits
                    nc.scalar.activation(
                        ind[:, c * kchunk : (c + 1) * kchunk],
                        ps,
                        AF.Sign,
                        bias=biasT[:, t : t + 1],
                        scale=1.0,
                    )
                rl = wp.tile([P, w], F32, tag="rl")
                nc.scalar.activation(
                    rl, ind, AF.Relu, accum_out=cnt[:, t : t + 1]
                )
                nc.vector.max_index(idx[:, t, :], ones8, ind)

            # ---- epilogue --------------------------------------------------
            idxf = cp.tile([P, NT, 8], F32)
            nc.vector.tensor_copy(idxf, idx)          # u32 -> f32 (exact)
            outf = cp.tile([P, NT, NS], F32)
            pred = cp.tile([P, NT], I32)
            for j in range(NS):
                nc.vector.tensor_copy(outf[:, :, j], idxf[:, :, 0])
                if j > 0:
                    nc.vector.tensor_scalar(
                        pred, cnt, float(j), None, op0=OP.is_gt
                    )
                    nc.vector.copy_predicated(
                        outf[:, :, j], pred, idxf[:, :, j]
                    )
            outi = cp.tile([P, NT, NS], I32)
            nc.vector.tensor_copy(outi, outf)         # f32 -> int32

            nc.sync.dma_start(
                out=out_d.rearrange("(t p) j -> p t j", p=P), in_=outi
            )
            nc.sync.dma_start(out=cnt_d, in_=cnt)
    nc.compile()
    return nc


_cache: dict[int, bass.Bass] = {}


def _get(w: int) -> bass.Bass:
    if w not in _cache:
        _cache[w] = _build(w)
    return _cache[w]


def _run(nc: bass.Bass, xs: list[np.ndarray], **kw):
    maps = [{"x": np.ascontiguousarray(xb, dtype=np.float32)} for xb in xs]
    return run_bass_kernel_spmd(nc, maps, list(range(len(xs))), **kw)


def kernel(x: np.ndarray) -> np.ndarray:
    x = np.asarray(x)
    assert x.shape == (B, N, 3), x.shape
    res = _run(_get(W_FAST), [x[b] for b in range(B)])
    out = np.stack([res.results[b]["out"] for b in range(B)])
    cnts = np.stack([res.results[b]["cnt"] for b in range(B)])
    bad = [b for b in range(B) if cnts[b].min() < NS]
    if bad:  # some row had < 5 hits in the window: exact full-width rerun
        res2 = _run(_get(N), [x[b] for b in bad])
        for i, b in enumerate(bad):
            out[b] = res2.results[i]["out"]
    return out.astype(np.int32)


# revision 22
# speedup vs baseline: 2.4851x; 2.4851x over previous
"""Trainium2 Bass kernel for pointnet2-style ball_query (radius=3.4, nsample=5).

Input : x [8, 4096, 3] f32.
Output: [8, 4096, 5] int32 - for each query q the first 5 point indices k (in
scan order) with ||x_q - x_k||^2 < r^2; missing slots hold the first hit.

Strategy (data-parallel, one batch per NeuronCore):
  - One K=4 PE matmul per 128-query tile computes
      ps[q,k] = <x_q, x_k> - sq_k/2
    over a W-column window into PSUM (lhsT = [x^T; 1], rhs = [x^T; -sq/2]).
  - ACT evacuates PSUM with Sign(ps + (r^2 - sq_q)/2) via a per-partition
    bias: the hit indicator in {-1, 0, +1}; accum_out gives S = hits - misses.
  - One DVE max_index matching eight 1.0s returns the first 8 hit positions
    per row in scan order - the whole selection in a single instruction.
  - Tiny epilogue: slot j (j>0) falls back to the first hit when count <= j.
Rows are only correct if they have >= 5 hits inside the window; the host
re-runs a full-width (W=4096) variant for any batch where some row's count
is below a safety margin (never happens for this data distribution: the
minimum window hit count is 13 at W=256).

Host-side work is restricted to pure layout permutations of x (transpose /
tile-major reshape) and of the output; all arithmetic runs on device.
"""

import numpy as np

import concourse.bass as bass
import concourse.bacc as bacc
import concourse.mybir as mybir
from concourse.tile import TileContext
from concourse.bass_utils import run_bass_kernel_spmd

N = 4096          # points per batch
B = 8             # batches == cores
P = 128           # partitions (query tile height)
NT = N // P       # 32 query tiles
NS = 5            # nsample
W_FAST = 256      # scan window of the fast kernel (min hits on data: 13)
CNT_MARGIN = 8    # fallback safety margin on the recovered hit count
R2 = float(np.float32(3.4 * 3.4))

F32 = mybir.dt.float32
I32 = mybir.dt.int32
U32 = mybir.dt.uint32
AF = mybir.ActivationFunctionType
OP = mybir.AluOpType


def _build(w: int) -> bass.Bass:
    """Build the single-core program scanning the first `w` columns."""
    assert w % P == 0
    kchunk = min(w, 512)             # PSUM tile width (one bank = 512 f32)
    nk = w // kchunk

    nc = bacc.Bacc("TRN2", target_bir_lowering=False, debug=False)
    # x in original layout (only the first w rows are read, for sq_k)
    x_in = nc.dram_tensor("x", [N, 3], F32, kind="ExternalInput").ap()
    # [x^T; ones] : host-side layout permutation of x
    xa_in = nc.dram_tensor("xa", [4, N], F32, kind="ExternalInput").ap()
    # query-tile-major x: xqh[p, 3*t+d] = x[t*128+p, d]
    xqh_in = nc.dram_tensor("xqh", [P, NT * 3], F32, kind="ExternalInput").ap()
    # outputs in device layout; host unpermutes
    out_d = nc.dram_tensor("out", [P, NT, NS], I32, kind="ExternalOutput").ap()
    cnt_d = nc.dram_tensor("cnt", [P, NT], F32, kind="ExternalOutput").ap()

    with TileContext(nc) as tc:
        with (
            tc.tile_pool(name="const", bufs=1) as cp,
            tc.tile_pool(name="psum", bufs=8, space="PSUM") as pp,
            tc.tile_pool(name="work", bufs=6 if w <= 512 else 2) as wp,
        ):
            # ---- setup -----------------------------------------------------
            A4 = cp.tile([4, N], F32)        # lhsT: [x^T; 1]
            nc.sync.dma_start(out=A4, in_=xa_in)
            xq = cp.tile([P, NT, 3], F32)
            nc.sync.dma_start(out=xq, in_=xqh_in.rearrange("p (t d) -> p t d", d=3))


            # sq[p, t] = |x_q|^2 for q = t*128+p
            xsq = cp.tile([P, NT, 3], F32)
            nc.scalar.activation(xsq, xq, AF.Square)
            sqt = cp.tile([P, NT], F32)
            nc.vector.tensor_add(sqt, xsq[:, :, 0], xsq[:, :, 1])
            nc.vector.tensor_add(sqt, sqt, xsq[:, :, 2])
            # bias2[p, t] = (r^2 - sq_q) / 2  (per-partition ACT bias)
            biasT = cp.tile([P, NT], F32)
            nc.vector.tensor_scalar(biasT, sqt, -0.5, 0.5 * R2, op0=OP.mult, op1=OP.add)

            # msqrow[0, k] = -sq_k / 2 for k < w (x loaded chunk-wise on one
            # partition, row layout, for the sq_k row)
            xrsq = cp.tile([1, kchunk, 3], F32)
            msqrow = cp.tile([1, w], F32)
            for c in range(nk):
                ksl = slice(c * kchunk, (c + 1) * kchunk)
                xrow = wp.tile([1, kchunk, 3], F32, tag="xrow")
                nc.sync.dma_start(
                    out=xrow,
                    in_=x_in[c * kchunk : (c + 1) * kchunk, :].rearrange(
                        "k d -> (k d)"
                    ),
                )
                nc.scalar.activation(xrsq, xrow, AF.Square)
                nc.vector.tensor_add(msqrow[:, ksl], xrsq[:, :, 0], xrsq[:, :, 1])
                nc.vector.tensor_add(msqrow[:, ksl], msqrow[:, ksl], xrsq[:, :, 2])
            nc.vector.tensor_scalar_mul(msqrow, msqrow, -0.5)

            # rhs B4[4, k] = [x^T; -sq/2] - row 3 written via DMA (engines
            # cannot start at partition 3, DMA can)
            B4 = cp.tile([4, w], F32)
            nc.sync.dma_start(out=B4[0:3, :], in_=xa_in[0:3, 0:w])
            nc.sync.dma_start(out=B4[3:4, :], in_=msqrow)

            ones8 = cp.tile([P, 8], F32)
            nc.vector.memset(ones8, 1.0)

            idx = cp.tile([P, NT, 8], U32)   # first-8 hit positions per row
            acc = cp.tile([P, NT, nk], F32)  # per-chunk sign-sums

            # ---- main loop: one 128-query tile at a time -------------------
            for t in range(NT):
                ind = wp.tile([P, w], F32, tag="ind")
                for c in range(nk):
                    ps = pp.tile([P, kchunk], F32, tag="ps")
                    ksl = slice(c * kchunk, (c + 1) * kchunk)
                    # ps = <x_q, x_k> - sq_k/2
                    nc.tensor.matmul(
                        ps,
                        A4[:, t * P : (t + 1) * P],
                        B4[:, ksl],
                        start=True,
                        stop=True,
                    )
                    # ind = sign(<x_q,x_k> - sq_k/2 + (r^2 - sq_q)/2)
                    #     = sign((r^2 - d2)/2) : +1 exactly at hits
                    nc.scalar.activation(
                        ind[:, ksl],
                        ps,
                        AF.Sign,
                        bias=biasT[:, t : t + 1],
                        scale=1.0,
                        accum_out=acc[:, t, c : c + 1],
                    )
                nc.vector.max_index(idx[:, t, :], ones8, ind)

            # ---- epilogue --------------------------------------------------
            # hit count h = (S + w) / 2  (exact when no d2 == r^2 ties; the
            # host fallback margin covers the pathological tie case)
            if nk == 1:
                accs = acc.rearrange("p t one -> p (t one)")
            else:
                accs = cp.tile([P, NT], F32)
                nc.vector.reduce_sum(accs, acc, axis=mybir.AxisListType.X)
            cnt = cp.tile([P, NT], F32)
            nc.vector.tensor_scalar(
                cnt, accs, float(w), 0.5, op0=OP.add, op1=OP.mult
            )
            idxf = cp.tile([P, NT, 8], F32)
            nc.vector.tensor_copy(idxf, idx)          # u32 -> f32 (exact)
            outf = cp.tile([P, NT, NS], F32)
            pred = cp.tile([P, NT], I32)
            for j in range(NS):
                nc.vector.tensor_copy(outf[:, :, j], idxf[:, :, 0])
                if j > 0:
                    nc.vector.tensor_scalar(
                        pred, cnt, float(j), None, op0=OP.is_gt
                    )
                    nc.vector.copy_predicated(
                        outf[:, :, j], pred, idxf[:, :, j]
                    )
            outi = cp.tile([P, NT, NS], I32)
            nc.vector.tensor_copy(outi, outf)         # f32 -> int32

            nc.sync.dma_start(out=out_d, in_=outi)
            nc.sync.dma_start(out=cnt_d, in_=cnt)
    nc.compile()
    return nc


_cache: dict[int, bass.Bass] = {}


def _get(w: int) -> bass.Bass:
    if w not in _cache:
        _cache[w] = _build(w)
    return _cache[w]


def _in_map(xb: np.ndarray) -> dict[str, np.ndarray]:
    xb = np.ascontiguousarray(xb, dtype=np.float32)
    xa = np.empty((4, N), np.float32)
    xa[0:3] = xb.T
    xa[3] = 1.0
    xqh = np.ascontiguousarray(
        xb.reshape(NT, P, 3).transpose(1, 0, 2).reshape(P, NT * 3)
    )
    return {"x": xb, "xa": xa, "xqh": xqh}


def _run(nc: bass.Bass, xs: list[np.ndarray], **kw):
    return run_bass_kernel_spmd(nc, [_in_map(xb) for xb in xs],
                                list(range(len(xs))), **kw)


def _unpermute(out_dev: np.ndarray) -> np.ndarray:
    # [P, NT, NS] with q = t*128 + p  ->  [N, NS]
    return out_dev.transpose(1, 0, 2).reshape(N, NS)


def kernel(x: np.ndarray) -> np.ndarray:
    x = np.asarray(x)
    assert x.shape == (B, N, 3), x.shape
    res = _run(_get(W_FAST), [x[b] for b in range(B)])
    out = np.stack([_unpermute(res.results[b]["out"]) for b in range(B)])
    cnts = np.stack([res.results[b]["cnt"] for b in range(B)])
    bad = [b for b in range(B) if cnts[b].min() < NS + CNT_MARGIN]
    if bad:  # some row too close to < 5 window hits: exact full-width rerun
        res2 = _run(_get(N), [x[b] for b in bad])
        for i, b in enumerate(bad):
            out[b] = _unpermute(res2.results[i]["out"])
    return out.astype(np.int32)
